# revision 7
# baseline (speedup 1.0000x reference)
"""Trainium2 Bass kernel for blended-expert MLP (moe_routing).

Model: for each of 3 layers, w_l = sum_e c_e * W[l,e]  (E=8 experts),
x = act(x @ w_l.T + B_l), act = ELU for layers 0,1, none for layer 2.

Strategy (8 NeuronCores):
- Data-parallel over the 4096-row batch (512 rows/core).
- The expert blend is sharded over the contraction (input) dim: core k blends
  i-slice k (256 rows of w_l.T) on the Vector engine in bf16 (2x DVE rate).
- Per layer, TWO column-halved AllGathers assemble w_l.T as [2048, 1024]
  column blocks in DRAM.  Each column block feeds exactly one PSUM-half of
  the matmul (full k-loop over 16 k-tiles), so the collective pipeline
  AG(l,c0) -> mm(l,half0), AG(l,c1) -> mm(l,half1) runs with no mid-layer
  dependency stalls and the 6 AGs hide behind the matmul stream.
- Matmuls run in bf16 (fp32 PSUM).  Activations stay SBUF-resident between
  layers in [feature, batch] orientation; ELU is fused DVE/ACT ops:
  max(exp(min(z,0))-1, z).  Output is written bf16 and upcast on host.
"""

import numpy as np

import concourse.mybir as mybir
import concourse.tile as tile
from concourse import bacc
from concourse.bass_utils import run_bass_kernel_spmd

N_CORES = 8
L = 3          # layers
E = 8          # experts
D = 2048       # feature dim
BATCH = 4096
BS = BATCH // N_CORES   # 512 batch rows per core
IS = D // N_CORES       # 256 contraction rows blended per core
KT = D // 128           # 16 k-tiles
MT = D // 128            # 16 m-tiles (output feature tiles)
HALF_M = MT // 2         # 8 psum banks per half
HC = D // 2              # 1024 columns per AG column-chunk

f32 = mybir.dt.float32
bf16 = mybir.dt.bfloat16

_cache: dict = {}


def _build():
    nc = bacc.Bacc("TRN2", target_bir_lowering=False, debug=False,
                   num_devices=N_CORES)
    # Per-core inputs (pre-sharded/transposed by host):
    # WtT: (L, E, IS, D) = this core's i-slice of W transposed to [in, out]
    WtT = nc.dram_tensor("WtT", [L, E, IS, D], bf16, kind="ExternalInput")
    # xT: (D, BS) = this core's batch shard, transposed
    xT = nc.dram_tensor("xT", [D, BS], bf16, kind="ExternalInput")
    # cb: (128, E) = weight_blend broadcast across partitions
    cb = nc.dram_tensor("cb", [128, E], f32, kind="ExternalInput")
    # Bp: (128, L*MT) = biases; col l*MT+m holds B[l, m*128 : (m+1)*128]
    Bp = nc.dram_tensor("Bp", [128, L * MT], f32, kind="ExternalInput")
    # Output: yT (D, BS) = this core's output shard, transposed, bf16
    yT = nc.dram_tensor("yT", [D, BS], bf16, kind="ExternalOutput")

    with tile.TileContext(nc) as tc:
        with (
            tc.tile_pool(name="const", bufs=1) as cpool,
            tc.tile_pool(name="acts", bufs=1) as apool,
            tc.tile_pool(name="blend", bufs=1) as bpool,
            tc.tile_pool(name="exp", bufs=16) as epool,
            tc.tile_pool(name="wst", bufs=12) as wpool,
            tc.tile_pool(name="epi", bufs=6) as xpool,
            tc.tile_pool(name="psum", bufs=1, space="PSUM") as ppool,
            tc.tile_pool(name="dram", bufs=1, space="DRAM") as dram,
        ):
            cbt = cpool.tile([128, E], f32)
            bt = cpool.tile([128, L * MT], f32)
            nc.sync.dma_start(cbt[:], cb[:])
            nc.sync.dma_start(bt[:], Bp[:])

            # Activations: two sets of KT tiles [128, BS], ping-pong.
            acts = [
                [apool.tile([128, BS], bf16, name=f"act{s}_{k}")
                 for k in range(KT)]
                for s in range(2)
            ]
            # DRAM bounce buffers for the column-chunked weight AllGathers:
            # agins[l][c] = my [IS, HC] column block; agouts[l][c] = the
            # assembled [D, HC] block (rows = w.T k-rows in natural order).
            agins = [[dram.tile([IS, HC], bf16, name=f"agin{l}_{c}")
                      for c in range(2)] for l in range(L)]
            agouts = [
                [dram.tile([D, HC], bf16, addr_space="Shared",
                           name=f"agout{l}_{c}") for c in range(2)]
                for l in range(L)
            ]

            # ---- blend + 2 column AllGathers per layer ----
            def emit_blend(l):
                for h in range(IS // 128):  # 2 half-slices of 128 partitions
                    acc = bpool.tile([128, D], bf16, name=f"acc{l}_{h}",
                                     tag=f"acc{h}", bufs=2)
                    for e in range(E):
                        et = epool.tile([128, D], bf16, name=f"exp{l}_{h}_{e}",
                                        tag="exp")
                        nc.scalar.dma_start(
                            et[:], WtT[l, e, h * 128:(h + 1) * 128, :]
                        )
                        if e == 0:
                            nc.vector.tensor_scalar_mul(
                                acc[:], et[:], cbt[:, 0:1]
                            )
                        else:
                            # acc = (et * c_e) + acc
                            nc.vector.scalar_tensor_tensor(
                                acc[:], et[:], cbt[:, e:e + 1], acc[:],
                                mybir.AluOpType.mult, mybir.AluOpType.add,
                            )
                    # scatter the two column halves to the AG inputs.
                    # gpsimd queue: these interleave between the collective
                    # triggers (which pace the same queue anyway) and stay
                    # off the ACT queue, whose epilogue ops free PSUM banks.
                    for c in range(2):
                        nc.gpsimd.dma_start(
                            agins[l][c][h * 128:(h + 1) * 128, :],
                            acc[:, c * HC:(c + 1) * HC],
                        )
                for c in range(2):
                    nc.gpsimd.collective_compute(
                        "AllGather", mybir.AluOpType.bypass,
                        ins=[agins[l][c].opt()], outs=[agouts[l][c].opt()],
                        replica_groups=[list(range(N_CORES))],
                    )

            emit_blend(0)

            for k in range(KT):
                nc.sync.dma_start(acts[0][k][:], xT[k * 128:(k + 1) * 128, :])

            emit_blend(1)
            emit_blend(2)

            for l in range(L):
                # ---- matmul: y_l.T[m,:] = sum_k w_l.T[k,m].T @ act[k] ----
                src = acts[l % 2]
                dst = acts[(l + 1) % 2]
                for half in range(2):
                    psums = [
                        ppool.tile([128, BS], f32,
                                   name=f"ps{l}_{half}_{m}", tag=f"bank{m}")
                        for m in range(HALF_M)
                    ]
                    for k in range(KT):
                        ws = wpool.tile([128, HC], bf16,
                                        name=f"ws{l}_{half}_{k}", tag="ws")
                        nc.sync.dma_start(
                            ws[:], agouts[l][half][k * 128:(k + 1) * 128, :]
                        )
                        for m in range(HALF_M):
                            nc.tensor.matmul(
                                psums[m][:],
                                ws[:, m * 128:(m + 1) * 128],
                                src[k][:],
                                start=(k == 0),
                                stop=(k == KT - 1),
                            )
                    # ---- epilogue: bias (+ ELU), write next-layer acts ----
                    for m in range(HALF_M):
                        gm = half * HALF_M + m
                        bias = bt[:, l * MT + gm: l * MT + gm + 1]
                        ps = psums[m]
                        if l < L - 1:
                            tt = xpool.tile([128, BS], f32,
                                            name=f"t{l}_{gm}", tag="tmin")
                            zt = xpool.tile([128, BS], f32,
                                            name=f"z{l}_{gm}", tag="zbias")
                            ut = xpool.tile([128, BS], f32,
                                            name=f"u{l}_{gm}", tag="uexp")
                            # t = min(psum + bias, 0) on DVE
                            nc.vector.tensor_scalar(
                                tt[:], ps[:], bias, 0.0,
                                mybir.AluOpType.add, mybir.AluOpType.min,
                            )
                            # z = psum + bias on ACT
                            nc.scalar.activation(
                                zt[:], ps[:],
                                mybir.ActivationFunctionType.Identity,
                                bias=bias,
                            )
                            # u = exp(t) on ACT
                            nc.scalar.activation(
                                ut[:], tt[:], mybir.ActivationFunctionType.Exp
                            )
                            # act_next = max(u - 1, z) on DVE, in bf16
                            nc.vector.scalar_tensor_tensor(
                                dst[gm][:], ut[:], 1.0, zt[:],
                                mybir.AluOpType.subtract, mybir.AluOpType.max,
                            )
                        else:
                            ot = xpool.tile([128, BS], bf16,
                                            name=f"o{gm}", tag="outt")
                            nc.scalar.activation(
                                ot[:], ps[:],
                                mybir.ActivationFunctionType.Identity,
                                bias=bias,
                            )
                            nc.gpsimd.dma_start(
                                yT[gm * 128:(gm + 1) * 128, :], ot[:]
                            )
    nc.finalize()
    return nc


def _get_nc():
    if "nc" not in _cache:
        _cache["nc"] = _build()
    return _cache["nc"]


def make_in_maps(weight_blend, x, W, B):
    weight_blend = np.asarray(weight_blend, dtype=np.float32)
    x = np.asarray(x, dtype=np.float32)
    W = np.asarray(W, dtype=np.float32)
    B = np.asarray(B, dtype=np.float32)

    cb = np.ascontiguousarray(np.broadcast_to(weight_blend[None, :], (128, E)))
    # Bp[p, l*MT+m] = B[l, m*128+p]
    Bp = np.ascontiguousarray(
        B.reshape(L, MT, 128).transpose(2, 0, 1).reshape(128, L * MT)
    )

    import ml_dtypes
    in_maps = []
    for k in range(N_CORES):
        WtT = np.ascontiguousarray(
            W[:, :, :, k * IS:(k + 1) * IS].transpose(0, 1, 3, 2)
        ).astype(ml_dtypes.bfloat16)
        xTk = np.ascontiguousarray(
            x[k * BS:(k + 1) * BS, :].T
        ).astype(ml_dtypes.bfloat16)
        in_maps.append({"WtT": WtT, "xT": xTk, "cb": cb, "Bp": Bp})
    return in_maps


def kernel(weight_blend, x, W, B) -> np.ndarray:
    in_maps = make_in_maps(weight_blend, x, W, B)
    nc = _get_nc()
    last_err = None
    for attempt in range(3):
        try:
            res = run_bass_kernel_spmd(nc, in_maps,
                                       core_ids=list(range(N_CORES)))
            out = np.empty((BATCH, D), dtype=np.float32)
            for k in range(N_CORES):
                out[k * BS:(k + 1) * BS, :] = \
                    res.results[k]["yT"].T.astype(np.float32)
            if np.isfinite(out).all():
                return out
            last_err = RuntimeError("non-finite kernel output")
        except Exception as e:  # transient NRT/device wedge: retry
            last_err = e
        import time as _time
        _time.sleep(10 * (attempt + 1))
    raise last_err
